# revision 1
# baseline (speedup 1.0000x reference)
"""v3 reconstruction: fp16 pipeline, sequential phases (111 us, err 4.1e-4).

Single-head causal attention (B=8, T=2048, E=1024, H=64) on 8 trn2 cores,
data-parallel over batch. Everything stays transposed until the end:
  x --SWDGE cast DMA--> xn fp16 --PE matmul-vs-identity transpose--> xT
  qkT psum = [Wq|Wk]^T @ xT (q rows 0-63, k rows 64-127); kT via SBUF DMA
  vT = Wv^T @ xT; v1[s,65] = PE-transpose(vT) + ones column (rowsum trick)
  scoresT = kT_j^T @ qT; wT = exp(scores/8) on ACT; diag masked
  outT[65,t] += v1_j^T @ wT_j; normalize + PE-transpose + store f32
"""

import numpy as np

import concourse.bass as bass
import concourse.mybir as mybir
from concourse.tile import TileContext
from concourse.masks import make_identity, make_upper_triangular
from concourse.bass_utils import run_bass_kernel_spmd

B, T, E, H = 8, 2048, 1024, 64
NT = T // 128
NE = E // 128
F16 = mybir.dt.float16
F32 = mybir.dt.float32
SCALE = float(H) ** -0.5


def _split_excess_waits(nc: bass.Bass, cap: int = 1) -> int:
    n_split = 0
    for f in nc.m.functions:
        for bb in f.blocks:
            insts = list(bb.instructions)
            out = []
            dirty = False
            for inst in insts:
                si = inst.sync_info
                waits = list(si.on_wait) if si and si.on_wait else []
                if len(waits) > cap:
                    si.on_wait = waits[:cap]
                    for w in waits[cap:]:
                        nop = mybir.InstNoOp(
                            name=f"I-waitsplit-{n_split}", ins=[], outs=[]
                        )
                        nop.engine = inst.engine
                        nop.sync_info = mybir.SyncInfo(on_wait=[w], on_update=[])
                        out.append(nop)
                        n_split += 1
                    dirty = True
                out.append(inst)
            if dirty:
                bb.instructions = out
    return n_split


def build_nc(split_waits: bool = True) -> bass.Bass:
    nc = bass.Bass()
    x = nc.dram_tensor("x", [T, E], F32, kind="ExternalInput")
    wq = nc.dram_tensor("Wq", [E, H], F32, kind="ExternalInput")
    wk = nc.dram_tensor("Wk", [E, H], F32, kind="ExternalInput")
    wv = nc.dram_tensor("Wv", [E, H], F32, kind="ExternalInput")
    out = nc.dram_tensor("out", [T, H], F32, kind="ExternalOutput")
    x_ap, out_ap = x.ap(), out.ap()

    with TileContext(nc) as tc:
        with (
            tc.tile_pool(name="const", bufs=1) as cpool,
            tc.tile_pool(name="wts", bufs=8) as wpool,
            tc.tile_pool(name="xnat", bufs=8) as xnpool,
            tc.tile_pool(name="xT", bufs=8) as xtpool,
            tc.tile_pool(name="qkv", bufs=1) as qkvpool,
            tc.tile_pool(name="wT", bufs=3) as wtpool,
            tc.tile_pool(name="fin", bufs=4) as finpool,
        ):
            eye16 = cpool.tile([128, 128], F16, tag="eye16")
            make_identity(nc, eye16[:])
            eye32 = cpool.tile([128, 128], F32, tag="eye32")
            make_identity(nc, eye32[:])
            tri = cpool.tile([128, 128], F16, tag="tri")
            make_upper_triangular(nc, tri[:], val=1.0, diag=True)

            w32 = {}
            for nm, dram in (("q", wq), ("k", wk), ("v", wv)):
                t32 = wpool.tile(
                    [128, NE * H], F32, tag="w32", bufs=3, name=f"w32{nm}"
                )
                nc.sync.dma_start(
                    t32[:].rearrange("p (j h) -> p j h", h=H),
                    dram.ap().rearrange("(j p) h -> p j h", p=128),
                )
                w32[nm] = t32
            wqk_t = []
            wv_t = []
            for j in range(NE):
                wt = wpool.tile([128, 128], F16, tag="wqk")
                nc.vector.tensor_copy(wt[:, 0:64], w32["q"][:, H * j : H * j + H])
                nc.vector.tensor_copy(wt[:, 64:128], w32["k"][:, H * j : H * j + H])
                wqk_t.append(wt)
                vt = wpool.tile([128, 64], F16, tag="wv")
                nc.vector.tensor_copy(vt[:], w32["v"][:, H * j : H * j + H])
                wv_t.append(vt)

            qkT = qkvpool.tile([128, T], F16, tag="qkT")
            kT = qkvpool.tile([64, T], F16, tag="kT")
            vT = qkvpool.tile([64, T], F16, tag="vT")
            v1 = qkvpool.tile([128, NT * 65], F16, tag="v1")
            outT_sb = qkvpool.tile([65, T], F32, tag="outT")

            xT = [
                xtpool.tile([128, T], F16, tag="xT", name=f"xT{j}")
                for j in range(NE)
            ]

            # ---------- phase A ----------
            with tc.tile_pool(name="ps1", bufs=1, space="PSUM") as ps1:
                xn = []
                for i in range(NT):
                    t_ = xnpool.tile([128, E], F16, tag="xn", bufs=10, name=f"xn{i}")
                    nc.gpsimd.dma_start(t_[:], x_ap[128 * i : 128 * i + 128, :])
                    xn.append(t_)

                for i0 in range(0, NT, 4):
                    for j in range(NE):
                        pt = ps1.tile([128, 512], F32, tag="tp", bufs=3)
                        for ii in range(4):
                            nc.tensor.matmul(
                                pt[:, 128 * ii : 128 * ii + 128],
                                xn[i0 + ii][:, 128 * j : 128 * j + 128],
                                eye16[:],
                                start=True, stop=True,
                            )
                        nc.scalar.copy(xT[j][:, 128 * i0 : 128 * i0 + 512], pt[:])

                for c in range(4):
                    sl = slice(512 * c, 512 * c + 512)
                    pqk = ps1.tile([128, 512], F32, tag="pqk", bufs=2)
                    pv = ps1.tile([64, 512], F32, tag="pv", bufs=2)
                    for j in range(NE):
                        nc.tensor.matmul(
                            pqk[:], wqk_t[j][:], xT[j][:, sl],
                            start=(j == 0), stop=(j == NE - 1),
                        )
                        nc.tensor.matmul(
                            pv[:], wv_t[j][:], xT[j][:, sl],
                            start=(j == 0), stop=(j == NE - 1),
                        )
                    nc.vector.tensor_copy(qkT[:, sl], pqk[:])
                    nc.scalar.copy(vT[:, sl], pv[:])
                    nc.sync.dma_start(kT[:, sl], qkT[64:128, sl])

            # ---------- phase B ----------
            with tc.tile_pool(name="ps2", bufs=1, space="PSUM") as ps2:
                for i in range(NT):
                    pt = ps2.tile([128, 64], F32, tag="sc", bufs=2)
                    nc.tensor.matmul(
                        pt[:, 0:64], vT[:, 128 * i : 128 * i + 128],
                        eye16[0:64, 0:64], start=True, stop=True,
                    )
                    nc.vector.tensor_copy(v1[:, 65 * i : 65 * i + 64], pt[:, 0:64])
                    nc.gpsimd.memset(v1[:, 65 * i + 64 : 65 * i + 65], 1.0)

                outT_ps = [
                    ps2.tile([65, 512], F32, tag="ot", bufs=4, name=f"ot{c}")
                    for c in range(4)
                ]
                wTs = {}

                def emit_2ndmm(j):
                    s0 = 128 * j
                    for c in range(s0 // 512, 4):
                        t0 = max(512 * c, s0)
                        t1 = 512 * c + 512
                        pad = t0 - 512 * c
                        nc.tensor.matmul(
                            outT_ps[c][:, pad:512],
                            v1[:, 65 * j : 65 * j + 65],
                            wTs[j][:, t0 - s0 : t1 - s0],
                            start=(j == 0), stop=(j == 4 * c + 3),
                        )

                for j in range(NT):
                    s0 = 128 * j
                    Lj = T - s0
                    wT = wtpool.tile([128, Lj], F16, tag="wT", bufs=4, name=f"wT{j}")
                    wTs[j] = wT
                    for kc in range(s0 // 1024, 2):
                        t0 = max(1024 * kc, s0)
                        t1 = 1024 * kc + 1024
                        w = t1 - t0
                        ps = ps2.tile([128, w], F32, tag="sc", bufs=2)
                        off = 0
                        while off < w:
                            n = min(512, w - off)
                            nc.tensor.matmul(
                                ps[:, off : off + n],
                                kT[:, s0 : s0 + 128],
                                qkT[0:64, t0 + off : t0 + off + n],
                                start=True, stop=True,
                            )
                            off += n
                        nc.scalar.activation(
                            wT[:, t0 - s0 : t1 - s0], ps[:],
                            mybir.ActivationFunctionType.Exp, scale=SCALE,
                        )
                    nc.vector.tensor_mul(wT[:, 0:128], wT[:, 0:128], tri[:])
                    if j >= 1:
                        emit_2ndmm(j - 1)
                emit_2ndmm(NT - 1)
                for c in range(4):
                    nc.vector.tensor_copy(
                        outT_sb[:, 512 * c : 512 * c + 512], outT_ps[c][:]
                    )

                for i in range(NT):
                    pt = ps2.tile([128, 65], F32, tag="sc", bufs=2)
                    nc.tensor.transpose(
                        pt[:, 0:65],
                        outT_sb[:, 128 * i : 128 * i + 128],
                        eye32[0:65, 0:65],
                    )
                    rcp = finpool.tile([128, 1], F32, tag="rcp")
                    nc.vector.reciprocal(rcp[:], pt[:, 64:65])
                    ob = finpool.tile([128, 64], F32, tag="ob")
                    nc.vector.tensor_scalar_mul(ob[:], pt[:, 0:64], rcp[:])
                    nc.sync.dma_start(out_ap[128 * i : 128 * i + 128, :], ob[:])

    if split_waits:
        _split_excess_waits(nc)
    return nc


_NC_CACHE = None


def _get_nc() -> bass.Bass:
    global _NC_CACHE
    if _NC_CACHE is None:
        _NC_CACHE = build_nc()
    return _NC_CACHE


def kernel(x, Wq, Wk, Wv, **run_kwargs):
    nc = _get_nc()
    x = np.ascontiguousarray(x, dtype=np.float32)
    in_maps = [
        {
            "x": np.ascontiguousarray(x[b]),
            "Wq": np.ascontiguousarray(Wq, dtype=np.float32),
            "Wk": np.ascontiguousarray(Wk, dtype=np.float32),
            "Wv": np.ascontiguousarray(Wv, dtype=np.float32),
        }
        for b in range(B)
    ]
    res = run_bass_kernel_spmd(nc, in_maps, core_ids=list(range(B)), **run_kwargs)
    out = np.stack([res.results[b]["out"] for b in range(B)], axis=0)
    kernel.last_results = res
    return out



# revision 8
# speedup vs baseline: 1.1453x; 1.1453x over previous
"""v4: single-head causal attention (B=8, T=2048, E=1024, H=64) on 8 trn2
cores, data-parallel over batch.

Pipeline per core (natural-v formulation, no final transpose):
  x f32 --HWDGE chunk DMA--> xn --PE f32r transpose--> psum --cast copy-->
  xT bf16 (group tiles, [j, t] layout)
  qkT[128, T] = [Wq|Wk]^T @ xT  (bf16, packed q rows 0:64 / k rows 64:128)
  v[t, 64]    = xT_chunk^T @ Wv (natural layout, ones col appended -> v1)
  scoresT[s-chunk, t] = kT_j^T @ qT  (bf16, diagonal-trimmed)
  wT = exp(scores/8) on ACT (psum -> sbuf bf16), diag masked by tri
  out_psum[t, 65] += wT_j^T @ v1_j   (col 64 = softmax denominator)
  out = psum[:, 0:64] * 1/psum[:, 64] on DVE -> staged -> DMA out

Emission is software-pipelined: loop g interleaves scores/exp(g),
AV(g-1), transposes(g+1), and projections(g+1) to keep PE dense (DVFS).
"""

import numpy as np

import concourse.bass as bass
import concourse.mybir as mybir
from concourse.tile import TileContext
from concourse.masks import make_identity, make_upper_triangular
from concourse.bass_utils import run_bass_kernel_spmd

B, T, E, H = 8, 2048, 1024, 64
NT = T // 128   # 16 t-chunks
NE = E // 128   # 8 e-chunks
NG = 4          # t-groups of 4 chunks / 512 cols
F32 = mybir.dt.float32
F32R = mybir.dt.float32r
BF16 = mybir.dt.bfloat16
SCALE = float(H) ** -0.5


def _split_excess_waits(nc: bass.Bass, cap: int = 1) -> int:
    n_split = 0
    for f in nc.m.functions:
        for bb in f.blocks:
            insts = list(bb.instructions)
            out = []
            dirty = False
            for inst in insts:
                si = inst.sync_info
                waits = list(si.on_wait) if si and si.on_wait else []
                if len(waits) > cap:
                    si.on_wait = waits[:cap]
                    for w in waits[cap:]:
                        nop = mybir.InstNoOp(
                            name=f"I-waitsplit-{n_split}", ins=[], outs=[]
                        )
                        nop.engine = inst.engine
                        nop.sync_info = mybir.SyncInfo(on_wait=[w], on_update=[])
                        out.append(nop)
                        n_split += 1
                    dirty = True
                out.append(inst)
            if dirty:
                bb.instructions = out
    return n_split


def build_nc(split_waits: bool = True) -> bass.Bass:
    nc = bass.Bass()
    x = nc.dram_tensor("x", [T, E], F32R, kind="ExternalInput")
    wq = nc.dram_tensor("Wq", [E, H], F32, kind="ExternalInput")
    wk = nc.dram_tensor("Wk", [E, H], F32, kind="ExternalInput")
    wv = nc.dram_tensor("Wv", [E, H], F32, kind="ExternalInput")
    out = nc.dram_tensor("out", [T, H], F32, kind="ExternalOutput")
    x_ap, out_ap = x.ap(), out.ap()

    with TileContext(nc) as tc:
        with (
            tc.tile_pool(name="const", bufs=1) as cpool,
            tc.tile_pool(name="wts", bufs=1) as wpool,
            tc.tile_pool(name="xn", bufs=8) as xnpool,
            tc.tile_pool(name="xtg", bufs=2) as xtpool,
            tc.tile_pool(name="qkv", bufs=1) as qkvpool,
            tc.tile_pool(name="wT", bufs=16) as wtpool,
            tc.tile_pool(name="fin", bufs=8) as finpool,
            tc.tile_pool(name="pT", bufs=2, space="PSUM") as pT,
            tc.tile_pool(name="pQK", bufs=1, space="PSUM") as pQK,
            tc.tile_pool(name="pV", bufs=1, space="PSUM") as pV,
            tc.tile_pool(name="pS", bufs=2, space="PSUM") as pS,
            tc.tile_pool(name="pAV", bufs=2, space="PSUM") as pAV,
        ):
            # ---- input DMAs first: x chunks on the sync (SP) HWDGE ring ----
            xn = []
            for i in range(NT):
                t_ = xnpool.tile([128, E], F32R, tag="xn", name=f"xn{i}")
                nc.sync.dma_start(t_[:], x_ap[128 * i : 128 * i + 128, :])
                xn.append(t_)
            # weights on the scalar (ACT) HWDGE ring
            w32 = {}
            for nm, dram in (("q", wq), ("k", wk), ("v", wv)):
                t32 = wpool.tile([128, NE * H], F32, tag=f"w32{nm}")
                nc.scalar.dma_start(
                    t32[:].rearrange("p (j h) -> p j h", h=H),
                    dram.ap().rearrange("(j p) h -> p j h", p=128),
                )
                w32[nm] = t32

            # ---- constants ----
            eye32 = cpool.tile([128, 128], F32, tag="eye32")
            make_identity(nc, eye32[:])
            eye_r = cpool.tile([128, 128], F32R, tag="eye_r")
            nc.vector.tensor_copy(eye_r[:], eye32[:])
            tri = cpool.tile([128, 128], BF16, tag="tri")
            make_upper_triangular(nc, tri[:], val=1.0, diag=True)

            # packed [Wq|Wk] lhsT tiles (bf16) and Wv (bf16)
            wqk = wpool.tile([128, NE * 128], BF16, tag="wqk")
            wvb = wpool.tile([128, NE * H], BF16, tag="wvb")
            for j in range(NE):
                nc.vector.tensor_copy(
                    wqk[:, 128 * j : 128 * j + 64],
                    w32["q"][:, H * j : H * j + H],
                )
                nc.vector.tensor_copy(
                    wqk[:, 128 * j + 64 : 128 * j + 128],
                    w32["k"][:, H * j : H * j + H],
                )
            nc.vector.tensor_copy(wvb[:], w32["v"][:])

            # ---- persistent tiles ----
            qTt = qkvpool.tile([64, T], BF16, tag="qTt")
            kTt = qkvpool.tile([64, T], BF16, tag="kTt")
            v1 = qkvpool.tile([128, NT * 65], BF16, tag="v1")
            # ones columns of v1 (col 64 of each 65-block)
            nc.vector.memset(
                v1[:].rearrange("p (i c) -> p i c", c=65)[:, :, 64:65], 1.0
            )
            xtg = [
                xtpool.tile([128, NE * 512], BF16, tag="xtg", name=f"xtg{g}")
                for g in range(NG)
            ]
            wT = [
                wtpool.tile([128, T], BF16, tag="wT", name=f"wT{j}")
                for j in range(NT)
            ]
            stage = [
                finpool.tile([128, 256], F32, tag="stage", bufs=2, name=f"st{g}")
                for g in range(NG)
            ]

            copy_engines = [nc.vector, nc.scalar]

            # ---------- emission unit generators ----------
            def t_units(g):
                """Transpose the 4 x-chunks of group g into xtg[g] (bf16)."""
                for c in range(4):
                    i = 4 * g + c
                    for jq in range(2):
                        def unit(i=i, c=c, jq=jq):
                            pt = pT.tile([128, 512], F32R, tag="pt")
                            for m in range(4):
                                j = 4 * jq + m
                                nc.tensor.transpose(
                                    pt[:, 128 * m : 128 * m + 128],
                                    xn[i][:, 128 * j : 128 * j + 128],
                                    eye_r[:],
                                )
                            # dest: 4 j-blocks at cols j*512 + c*128
                            dst = (
                                xtg[i // 4][:]
                                .rearrange("p (j t) -> p j t", t=512)[
                                    :, 4 * jq : 4 * jq + 4,
                                    128 * c : 128 * c + 128,
                                ]
                            )
                            eng = copy_engines[(2 * i + jq) % 2]
                            if eng is nc.scalar:
                                nc.scalar.copy(dst, pt[:].rearrange(
                                    "p (j t) -> p j t", t=128))
                            else:
                                nc.vector.tensor_copy(dst, pt[:].rearrange(
                                    "p (j t) -> p j t", t=128))
                        yield unit

            def qkvn_units(g):
                """qk-projection and v-projection for group g."""
                def qk_unit(jpair):
                    def unit():
                        if jpair == 0:
                            qkvn_units.pqk = pQK.tile([128, 512], F32, tag="pqk")
                        pqk = qkvn_units.pqk
                        for j in (2 * jpair, 2 * jpair + 1):
                            nc.tensor.matmul(
                                pqk[:],
                                wqk[:, 128 * j : 128 * j + 128],
                                xtg[g][:, 512 * j : 512 * j + 512],
                                start=(j == 0),
                                stop=(j == NE - 1),
                            )
                        if jpair == 3:
                            nc.vector.tensor_copy(
                                qTt[:, 512 * g : 512 * g + 512], pqk[0:64, :]
                            )
                            nc.scalar.copy(
                                kTt[:, 512 * g : 512 * g + 512], pqk[64:128, :]
                            )
                    return unit
                for jp in range(4):
                    yield qk_unit(jp)

                def v_unit(c):
                    def unit():
                        if c == 0:
                            qkvn_units.pv = pV.tile([128, 256], F32, tag="pv")
                        pv = qkvn_units.pv
                        for j in range(NE):
                            nc.tensor.matmul(
                                pv[:, 64 * c : 64 * c + 64],
                                xtg[g][:, 512 * j + 128 * c : 512 * j + 128 * c + 128],
                                wvb[:, 64 * j : 64 * j + 64],
                                start=(j == 0),
                                stop=(j == NE - 1),
                            )
                        if c == 3:
                            i0 = 4 * g
                            dst = (
                                v1[:]
                                .rearrange("p (i c) -> p i c", c=65)[
                                    :, i0 : i0 + 4, 0:64
                                ]
                            )
                            nc.scalar.copy(
                                dst, pv[:].rearrange("p (i c) -> p i c", c=64)
                            )
                    return unit
                for c in range(4):
                    yield v_unit(c)

            def s_units(g):
                """scoresT + exp for all s-chunks j <= 4g+3 over t-block g."""
                for j in range(4 * g + 4):
                    def unit(j=j, g=g):
                        off = max(0, 128 * j - 512 * g)  # diagonal trim
                        ps = pS.tile([128, 512], F32, tag="ps")
                        o = off
                        while o < 512:
                            n = min(512 - o, 512)
                            nc.tensor.matmul(
                                ps[:, o : o + n],
                                kTt[:, 128 * j : 128 * j + 128],
                                qTt[:, 512 * g + o : 512 * g + o + n],
                                start=True,
                                stop=True,
                            )
                            o += n
                        nc.scalar.activation(
                            wT[j][:, 512 * g + off : 512 * g + 512],
                            ps[:, off:512],
                            mybir.ActivationFunctionType.Exp,
                            scale=SCALE,
                        )
                        if 4 * g <= j:  # diagonal block: mask after exp
                            nc.vector.tensor_mul(
                                wT[j][:, 128 * j : 128 * j + 128],
                                wT[j][:, 128 * j : 128 * j + 128],
                                tri[:],
                            )
                    yield unit

            def av_units(g):
                """AV accumulation for the 4 t-chunks of group g + norm."""
                for c in range(4):
                    i = 4 * g + c
                    js = list(range(i + 1))
                    batches = [js[k : k + 4] for k in range(0, len(js), 4)]
                    for bi, batch in enumerate(batches):
                        def unit(i=i, c=c, g=g, bi=bi, batch=batch,
                                 last=(bi == len(batches) - 1)):
                            if i % 4 == 0 and bi == 0:
                                av_units.pav = pAV.tile(
                                    [128, 260], F32, tag="pav"
                                )
                            pav = av_units.pav
                            for j in batch:
                                nc.tensor.matmul(
                                    pav[:, 65 * c : 65 * c + 65],
                                    wT[j][:, 128 * i : 128 * i + 128],
                                    v1[:, 65 * j : 65 * j + 65],
                                    start=(j == 0),
                                    stop=(j == i),
                                )
                            if last:
                                rcp = finpool.tile([128, 1], F32, tag="rcp",
                                                   bufs=4)
                                nc.vector.reciprocal(
                                    rcp[:], pav[:, 65 * c + 64 : 65 * c + 65]
                                )
                                nc.vector.tensor_scalar_mul(
                                    stage[g][:, 64 * c : 64 * c + 64],
                                    pav[:, 65 * c : 65 * c + 64],
                                    rcp[:],
                                )
                                if c == 3:
                                    nc.scalar.dma_start(
                                        out_ap[512 * g : 512 * g + 512, :]
                                        .rearrange("(c p) h -> p c h", p=128),
                                        stage[g][:].rearrange(
                                            "p (c h) -> p c h", h=64
                                        ),
                                    )
                        yield unit

            def drain(*streams):
                streams = [s for s in streams if s is not None]
                while streams:
                    nxt = []
                    for s in streams:
                        u = next(s, None)
                        if u is not None:
                            u()
                            nxt.append(s)
                    streams = nxt

            # ---------- prologue: group 0 transpose + projections ----------
            drain(t_units(0))
            drain(qkvn_units(0))

            # ---------- steady loop ----------
            for g in range(NG):
                a = s_units(g)
                b = av_units(g - 1) if g >= 1 else None
                c = t_units(g + 1) if g + 1 < NG else None
                d = qkvn_units(g + 1) if g + 1 < NG else None
                # round-robin a/b with double-weight c; d after c drains
                streams = [s for s in (a, b) if s is not None]
                cd = c
                c_done = False
                while streams or cd is not None:
                    nxt = []
                    for s in streams:
                        u = next(s, None)
                        if u is not None:
                            u()
                            nxt.append(s)
                    if cd is not None:
                        for _ in range(2):
                            u = next(cd, None)
                            if u is None:
                                if not c_done:
                                    c_done = True
                                    cd = d
                                else:
                                    cd = None
                                break
                            u()
                    streams = nxt

            # ---------- epilogue: AV of the last group ----------
            drain(av_units(NG - 1))

    if split_waits:
        _split_excess_waits(nc)
    return nc


_NC_CACHE = None


def _get_nc() -> bass.Bass:
    global _NC_CACHE
    if _NC_CACHE is None:
        _NC_CACHE = build_nc()
    return _NC_CACHE


def kernel(x, Wq, Wk, Wv, **run_kwargs):
    nc = _get_nc()
    x = np.ascontiguousarray(x, dtype=np.float32)
    in_maps = [
        {
            "x": np.ascontiguousarray(x[b]),
            "Wq": np.ascontiguousarray(Wq, dtype=np.float32),
            "Wk": np.ascontiguousarray(Wk, dtype=np.float32),
            "Wv": np.ascontiguousarray(Wv, dtype=np.float32),
        }
        for b in range(B)
    ]
    res = run_bass_kernel_spmd(nc, in_maps, core_ids=list(range(B)), **run_kwargs)
    out = np.stack([res.results[b]["out"] for b in range(B)], axis=0)
    kernel.last_results = res
    return out


# revision 10
# speedup vs baseline: 1.3668x; 1.1934x over previous
"""v4: single-head causal attention (B=8, T=2048, E=1024, H=64) on 8 trn2
cores, data-parallel over batch.

Pipeline per core (natural-v formulation, no final transpose):
  x f32 --HWDGE chunk DMA--> xn --PE f32r transpose--> psum --cast copy-->
  xT bf16 (group tiles, [j, t] layout)
  qkT[128, T] = [Wq|Wk]^T @ xT  (bf16, packed q rows 0:64 / k rows 64:128)
  v[t, 64]    = xT_chunk^T @ Wv (natural layout, ones col appended -> v1)
  scoresT[s-chunk, t] = kT_j^T @ qT  (bf16, diagonal-trimmed)
  wT = exp(scores/8) on ACT (psum -> sbuf bf16), diag masked by tri
  out_psum[t, 65] += wT_j^T @ v1_j   (col 64 = softmax denominator)
  out = psum[:, 0:64] * 1/psum[:, 64] on DVE -> staged -> DMA out

Emission is software-pipelined: loop g interleaves scores/exp(g),
AV(g-1), transposes(g+1), and projections(g+1) to keep PE dense (DVFS).
"""

import numpy as np

import concourse.bass as bass
import concourse.mybir as mybir
from concourse.tile import TileContext
from concourse.masks import make_identity, make_upper_triangular
from concourse.bass_utils import run_bass_kernel_spmd

B, T, E, H = 8, 2048, 1024, 64
NT = T // 128   # 16 t-chunks
NE = E // 128   # 8 e-chunks
NG = 4          # t-groups of 4 chunks / 512 cols
F32 = mybir.dt.float32
F32R = mybir.dt.float32r
BF16 = mybir.dt.bfloat16
SCALE = float(H) ** -0.5


def _split_excess_waits(nc: bass.Bass, cap: int = 1) -> int:
    n_split = 0
    for f in nc.m.functions:
        for bb in f.blocks:
            insts = list(bb.instructions)
            out = []
            dirty = False
            for inst in insts:
                si = inst.sync_info
                waits = list(si.on_wait) if si and si.on_wait else []
                if len(waits) > cap:
                    si.on_wait = waits[:cap]
                    for w in waits[cap:]:
                        nop = mybir.InstNoOp(
                            name=f"I-waitsplit-{n_split}", ins=[], outs=[]
                        )
                        nop.engine = inst.engine
                        nop.sync_info = mybir.SyncInfo(on_wait=[w], on_update=[])
                        out.append(nop)
                        n_split += 1
                    dirty = True
                out.append(inst)
            if dirty:
                bb.instructions = out
    return n_split


def build_nc(split_waits: bool = True) -> bass.Bass:
    nc = bass.Bass()
    x = nc.dram_tensor("x", [T, E], F32R, kind="ExternalInput")
    wq = nc.dram_tensor("Wq", [E, H], F32, kind="ExternalInput")
    wk = nc.dram_tensor("Wk", [E, H], F32, kind="ExternalInput")
    wv = nc.dram_tensor("Wv", [E, H], F32, kind="ExternalInput")
    out = nc.dram_tensor("out", [T, H], F32, kind="ExternalOutput")
    x_ap, out_ap = x.ap(), out.ap()

    with TileContext(nc) as tc:
        with (
            tc.tile_pool(name="const", bufs=1) as cpool,
            tc.tile_pool(name="wts", bufs=1) as wpool,
            tc.tile_pool(name="xn", bufs=8) as xnpool,
            tc.tile_pool(name="xtg", bufs=2) as xtpool,
            tc.tile_pool(name="qkv", bufs=1) as qkvpool,
            tc.tile_pool(name="wT", bufs=16) as wtpool,
            tc.tile_pool(name="fin", bufs=8) as finpool,
            tc.tile_pool(name="pT", bufs=2, space="PSUM") as pT,
            tc.tile_pool(name="pQK", bufs=1, space="PSUM") as pQK,
            tc.tile_pool(name="pV", bufs=1, space="PSUM") as pV,
            tc.tile_pool(name="pS", bufs=3, space="PSUM") as pS,
            tc.tile_pool(name="pAV", bufs=1, space="PSUM") as pAV,
        ):
            # ---- input DMAs first: x chunks on the sync (SP) HWDGE ring ----
            xn = []
            for i in range(NT):
                t_ = xnpool.tile([128, E], F32R, tag="xn", name=f"xn{i}")
                nc.sync.dma_start(t_[:], x_ap[128 * i : 128 * i + 128, :])
                xn.append(t_)
            # weights on the scalar (ACT) HWDGE ring
            w32 = {}
            for nm, dram in (("q", wq), ("k", wk), ("v", wv)):
                t32 = wpool.tile([128, NE * H], F32, tag=f"w32{nm}")
                nc.scalar.dma_start(
                    t32[:].rearrange("p (j h) -> p j h", h=H),
                    dram.ap().rearrange("(j p) h -> p j h", p=128),
                )
                w32[nm] = t32

            # ---- constants ----
            eye32 = cpool.tile([128, 128], F32, tag="eye32")
            make_identity(nc, eye32[:])
            eye_r = cpool.tile([128, 128], F32R, tag="eye_r")
            nc.vector.tensor_copy(eye_r[:], eye32[:])
            tri = cpool.tile([128, 128], BF16, tag="tri")
            make_upper_triangular(nc, tri[:], val=1.0, diag=True)

            # packed [Wq|Wk] lhsT tiles (bf16) and Wv (bf16)
            wqk = wpool.tile([128, NE * 128], BF16, tag="wqk")
            wvb = wpool.tile([128, NE * H], BF16, tag="wvb")
            for j in range(NE):
                nc.vector.tensor_copy(
                    wqk[:, 128 * j : 128 * j + 64],
                    w32["q"][:, H * j : H * j + H],
                )
                nc.vector.tensor_copy(
                    wqk[:, 128 * j + 64 : 128 * j + 128],
                    w32["k"][:, H * j : H * j + H],
                )
            nc.vector.tensor_copy(wvb[:], w32["v"][:])

            # ---- persistent tiles ----
            qTt = qkvpool.tile([64, T], BF16, tag="qTt")
            kTt = qkvpool.tile([64, T], BF16, tag="kTt")
            v1 = qkvpool.tile([128, NT * 65], BF16, tag="v1")
            # ones columns of v1 (col 64 of each 65-block)
            nc.vector.memset(
                v1[:].rearrange("p (i c) -> p i c", c=65)[:, :, 64:65], 1.0
            )
            xtg = [
                xtpool.tile([128, NE * 512], BF16, tag="xtg", name=f"xtg{g}")
                for g in range(NG)
            ]
            wT = [
                wtpool.tile([128, T], BF16, tag="wT", name=f"wT{j}")
                for j in range(NT)
            ]
            stage = [
                finpool.tile([128, 256], F32, tag="stage", bufs=2, name=f"st{g}")
                for g in range(NG)
            ]

            # ---------- emission unit generators ----------
            def t_units(g):
                """Transpose the 4 x-chunks of group g into xtg[g] (bf16)."""
                for c in range(4):
                    i = 4 * g + c
                    for jq in range(2):
                        def unit(i=i, c=c, jq=jq):
                            pt = pT.tile([128, 512], F32R, tag="pt")
                            for m in range(4):
                                j = 4 * jq + m
                                nc.tensor.transpose(
                                    pt[:, 128 * m : 128 * m + 128],
                                    xn[i][:, 128 * j : 128 * j + 128],
                                    eye_r[:],
                                )
                            # dest: 4 j-blocks at cols j*512 + c*128
                            dst = (
                                xtg[i // 4][:]
                                .rearrange("p (j t) -> p j t", t=512)[
                                    :, 4 * jq : 4 * jq + 4,
                                    128 * c : 128 * c + 128,
                                ]
                            )
                            nc.vector.tensor_copy(
                                dst, pt[:].rearrange("p (j t) -> p j t", t=128)
                            )
                        yield unit

            def qkvn_units(g):
                """qk-projection and v-projection for group g."""
                def qk_unit(jpair):
                    def unit():
                        if jpair == 0:
                            qkvn_units.pqk = pQK.tile([128, 512], F32, tag="pqk")
                        pqk = qkvn_units.pqk
                        for j in (2 * jpair, 2 * jpair + 1):
                            nc.tensor.matmul(
                                pqk[:],
                                wqk[:, 128 * j : 128 * j + 128],
                                xtg[g][:, 512 * j : 512 * j + 512],
                                start=(j == 0),
                                stop=(j == NE - 1),
                            )
                        if jpair == 3:
                            nc.vector.tensor_copy(
                                qTt[:, 512 * g : 512 * g + 512], pqk[0:64, :]
                            )
                            nc.scalar.copy(
                                kTt[:, 512 * g : 512 * g + 512], pqk[64:128, :]
                            )
                    return unit
                for jp in range(4):
                    yield qk_unit(jp)

                def v_unit(c):
                    def unit():
                        if c == 0:
                            qkvn_units.pv = pV.tile([128, 256], F32, tag="pv")
                        pv = qkvn_units.pv
                        for j in range(NE):
                            nc.tensor.matmul(
                                pv[:, 64 * c : 64 * c + 64],
                                xtg[g][:, 512 * j + 128 * c : 512 * j + 128 * c + 128],
                                wvb[:, 64 * j : 64 * j + 64],
                                start=(j == 0),
                                stop=(j == NE - 1),
                            )
                        if c == 3:
                            i0 = 4 * g
                            dst = (
                                v1[:]
                                .rearrange("p (i c) -> p i c", c=65)[
                                    :, i0 : i0 + 4, 0:64
                                ]
                            )
                            nc.scalar.copy(
                                dst, pv[:].rearrange("p (i c) -> p i c", c=64)
                            )
                    return unit
                for c in range(4):
                    yield v_unit(c)

            def s_units(g):
                """scoresT + exp for all s-chunks j <= 4g+3 over t-block g."""
                for j in range(4 * g + 4):
                    def unit(j=j, g=g):
                        off = max(0, 128 * j - 512 * g)  # diagonal trim
                        ps = pS.tile([128, 512], F32, tag="ps")
                        o = off
                        while o < 512:
                            n = min(512 - o, 512)
                            nc.tensor.matmul(
                                ps[:, o : o + n],
                                kTt[:, 128 * j : 128 * j + 128],
                                qTt[:, 512 * g + o : 512 * g + o + n],
                                start=True,
                                stop=True,
                            )
                            o += n
                        nc.scalar.activation(
                            wT[j][:, 512 * g + off : 512 * g + 512],
                            ps[:, off:512],
                            mybir.ActivationFunctionType.Exp,
                            scale=SCALE,
                        )
                        if 4 * g <= j:  # diagonal block: mask after exp
                            nc.vector.tensor_mul(
                                wT[j][:, 128 * j : 128 * j + 128],
                                wT[j][:, 128 * j : 128 * j + 128],
                                tri[:],
                            )
                    yield unit

            def av_units(g):
                """AV accumulation for the 4 t-chunks of group g (j-major:
                consecutive matmuls hit different psum col-slices so the
                accumulation chains pipeline) + normalization."""
                def alloc():
                    av_units.pav = pAV.tile([128, 260], F32, tag="pav")
                yield alloc
                for c in range(4):
                    i = 4 * g + c
                    js = list(range(i + 1))
                    batches = [js[k : k + 4] for k in range(0, len(js), 4)]
                    for batch in batches:
                        def unit(i=i, c=c, batch=batch):
                            pav = av_units.pav
                            for j in batch:
                                nc.tensor.matmul(
                                    pav[:, 65 * c : 65 * c + 65],
                                    wT[j][:, 128 * i : 128 * i + 128],
                                    v1[:, 65 * j : 65 * j + 65],
                                    start=(j == 0),
                                    stop=(j == i),
                                )
                        yield unit
                def norm(g=g):
                    pav = av_units.pav
                    rcp = finpool.tile([128, 4], F32, tag="rcp", bufs=2)
                    nc.vector.reciprocal(
                        rcp[:],
                        pav[:].rearrange("p (c x) -> p c x", x=65)[:, :, 64:65],
                    )
                    for c in range(4):
                        nc.vector.tensor_scalar_mul(
                            stage[g][:, 64 * c : 64 * c + 64],
                            pav[:, 65 * c : 65 * c + 64],
                            rcp[:, c : c + 1],
                        )
                    nc.scalar.dma_start(
                        out_ap[512 * g : 512 * g + 512, :]
                        .rearrange("(c p) h -> p c h", p=128),
                        stage[g][:].rearrange("p (c h) -> p c h", h=64),
                    )
                yield norm

            def drain(*streams):
                streams = [s for s in streams if s is not None]
                while streams:
                    nxt = []
                    for s in streams:
                        u = next(s, None)
                        if u is not None:
                            u()
                            nxt.append(s)
                    streams = nxt

            # ---------- prologue: group 0 transpose + projections ----------
            drain(t_units(0))
            drain(qkvn_units(0))

            # ---------- steady loop ----------
            for g in range(NG):
                a = s_units(g)
                b = av_units(g - 1) if g >= 1 else None
                c = t_units(g + 1) if g + 1 < NG else None
                d = qkvn_units(g + 1) if g + 1 < NG else None
                # round-robin a/b with double-weight c; d after c drains
                streams = [s for s in (a, b) if s is not None]
                cd = c
                c_done = False
                while streams or cd is not None:
                    nxt = []
                    for s in streams:
                        u = next(s, None)
                        if u is not None:
                            u()
                            nxt.append(s)
                    if cd is not None:
                        for _ in range(2):
                            u = next(cd, None)
                            if u is None:
                                if not c_done:
                                    c_done = True
                                    cd = d
                                else:
                                    cd = None
                                break
                            u()
                    streams = nxt

            # ---------- epilogue: AV of the last group ----------
            drain(av_units(NG - 1))

    if split_waits:
        _split_excess_waits(nc)
    return nc


_NC_CACHE = None


def _get_nc() -> bass.Bass:
    global _NC_CACHE
    if _NC_CACHE is None:
        _NC_CACHE = build_nc()
    return _NC_CACHE


def kernel(x, Wq, Wk, Wv, **run_kwargs):
    nc = _get_nc()
    x = np.ascontiguousarray(x, dtype=np.float32)
    in_maps = [
        {
            "x": np.ascontiguousarray(x[b]),
            "Wq": np.ascontiguousarray(Wq, dtype=np.float32),
            "Wk": np.ascontiguousarray(Wk, dtype=np.float32),
            "Wv": np.ascontiguousarray(Wv, dtype=np.float32),
        }
        for b in range(B)
    ]
    res = run_bass_kernel_spmd(nc, in_maps, core_ids=list(range(B)), **run_kwargs)
    out = np.stack([res.results[b]["out"] for b in range(B)], axis=0)
    kernel.last_results = res
    return out
